# revision 21
# baseline (speedup 1.0000x reference)
"""MoE Transformer Block kernel for Trainium2, 8 NeuronCores.

Sharding: attention head-parallel (2 heads/core), MoE expert-parallel
(1 expert/core, dense-expert V1), token-sliced LN/output (512 tok/core).
Collectives: AllToAll(ctx) -> AllGather(x1 bf16) -> AllGather(logits)
-> ReduceScatter(moe out).
"""
import sys, types

# ---- antenv.axon_hooks shim (image's antenv lacks this tiny registry) ----
def _install_hook_shim():
    try:
        import antenv
    except ImportError:
        return
    if "antenv.axon_hooks" in sys.modules:
        return
    m = types.ModuleType("antenv.axon_hooks")
    m._hook = None
    def _set(h): m._hook = h
    def _get(): return m._hook
    m.set_axon_ntff_profile_hook = _set
    m.get_axon_ntff_profile_hook = _get
    sys.modules["antenv.axon_hooks"] = m
    antenv.axon_hooks = m
    try:
        from trn_agent_boot.trn_boot import _ntff_profile_via_ctypes
        import os
        if os.path.exists("/opt/axon/libaxon_pjrt.so"):
            _set(_ntff_profile_via_ctypes("/opt/axon/libaxon_pjrt.so"))
    except Exception:
        pass

_install_hook_shim()

import numpy as np
import ml_dtypes
import concourse.bass as bass
import concourse.bacc as bacc
import concourse.tile as tile
from concourse import mybir
from concourse.bass_utils import run_bass_kernel_spmd

F32 = mybir.dt.float32
F32R = mybir.dt.float32r
BF16 = mybir.dt.bfloat16
AF = mybir.ActivationFunctionType
ALU = mybir.AluOpType

B, S, D, H, FF, E, TOPK = 2, 2048, 1024, 16, 4096, 8, 2
N = B * S          # 4096 tokens
NC = 8             # cores
TOK = N // NC      # 512 tokens per core
HPC = H // NC      # 2 heads per core
HD = D // H        # 64
EPS = 1e-5
P = 128

_CACHE = {}


def build_nc():
    nc = bacc.Bacc("TRN2", target_bir_lowering=False, debug=False, num_devices=NC)

    # ---------------- DRAM I/O ----------------
    xt = nc.dram_tensor("xt", [B, D, S], F32, kind="ExternalInput")       # x[b].T
    xs = nc.dram_tensor("xs", [TOK, D], F32, kind="ExternalInput")        # x slice
    wqt = nc.dram_tensor("wqt", [D, P], F32, kind="ExternalInput")
    wkt = nc.dram_tensor("wkt", [D, P], F32, kind="ExternalInput")
    wvt = nc.dram_tensor("wvt", [D, P], F32, kind="ExternalInput")
    bq = nc.dram_tensor("bq", [P, 1], F32, kind="ExternalInput")
    bk = nc.dram_tensor("bk", [P, 1], F32, kind="ExternalInput")
    bv = nc.dram_tensor("bv", [P, 1], F32, kind="ExternalInput")
    owt = nc.dram_tensor("owt", [D, D], F32, kind="ExternalInput")        # out_w.T
    ob128 = nc.dram_tensor("ob128", [P, D], F32, kind="ExternalInput")
    lg1 = nc.dram_tensor("lg1", [P, D], F32, kind="ExternalInput")
    lb1 = nc.dram_tensor("lb1", [P, D], F32, kind="ExternalInput")
    lg2 = nc.dram_tensor("lg2", [P, D], F32, kind="ExternalInput")
    lb2 = nc.dram_tensor("lb2", [P, D], F32, kind="ExternalInput")
    gwt = nc.dram_tensor("gwt", [D, E], F32, kind="ExternalInput")        # gate_w.T
    gb128 = nc.dram_tensor("gb128", [P, E], F32, kind="ExternalInput")
    w1t = nc.dram_tensor("w1t", [D, FF], BF16, kind="ExternalInput")      # w1[e].T
    b1e = nc.dram_tensor("b1e", [P, FF // P], F32, kind="ExternalInput")  # col ft
    w2t = nc.dram_tensor("w2t", [FF, D], BF16, kind="ExternalInput")      # w2[e].T
    b2e128 = nc.dram_tensor("b2e128", [P, D], F32, kind="ExternalInput")
    onehot = nc.dram_tensor("onehot", [P, E], F32, kind="ExternalInput")
    ident = nc.dram_tensor("ident", [P, P], F32, kind="ExternalInput")
    ones128 = nc.dram_tensor("ones128", [P, 1], F32, kind="ExternalInput")

    x2s = nc.dram_tensor("x2s", [TOK, D], F32, kind="ExternalOutput")
    lbo = nc.dram_tensor("lb", [1, 1], F32, kind="ExternalOutput")
    dbg_x1 = nc.dram_tensor("dbg_x1", [TOK, D], F32, kind="ExternalOutput")
    dbg_lg = nc.dram_tensor("dbg_lg", [TOK, E], F32, kind="ExternalOutput")

    with tile.TileContext(nc) as tc:
        _emit(nc, tc, locals())
    nc.compile()
    return nc



def _scope(nc, name):
    import contextlib
    es = contextlib.ExitStack()
    es.enter_context(nc.named_scope(name))
    return es

def _emit(nc, tc, t):
    xt, xs = t["xt"], t["xs"]
    QC = S // 512  # 4 q-chunks per batch

    with (
        tc.tile_pool(name="const", bufs=1) as cp,
        tc.tile_pool(name="persist", bufs=1) as pp,
        tc.tile_pool(name="dram", bufs=1, space="DRAM") as dp,
    ):
        # ---- constants ----
        wq = [cp.tile([P, P], F32R, tag=f"wq{k}", name=f"wq{k}") for k in range(8)]
        wk = [cp.tile([P, P], F32R, tag=f"wk{k}", name=f"wk{k}") for k in range(8)]
        wv = [cp.tile([P, P], F32R, tag=f"wv{k}", name=f"wv{k}") for k in range(8)]
        with tc.tile_pool(name="wstage", bufs=2) as stp:
            for k in range(8):
                wqf = stp.tile([P, P], F32, tag="wqf", name="wqf")
                wkf = stp.tile([P, P], F32, tag="wkf", name="wkf")
                wvf = stp.tile([P, P], F32, tag="wvf", name="wvf")
                nc.sync.dma_start(wqf[:], t["wqt"][k * P:(k + 1) * P, :])
                nc.sync.dma_start(wkf[:], t["wkt"][k * P:(k + 1) * P, :])
                nc.sync.dma_start(wvf[:], t["wvt"][k * P:(k + 1) * P, :])
                nc.vector.tensor_copy(wq[k][:], wqf[:])
                nc.vector.tensor_copy(wk[k][:], wkf[:])
                nc.vector.tensor_copy(wv[k][:], wvf[:])
        bq = cp.tile([P, 1], F32, tag="bq", name="bq"); nc.sync.dma_start(bq[:], t["bq"][:])
        bk = cp.tile([P, 1], F32, tag="bk", name="bk"); nc.sync.dma_start(bk[:], t["bk"][:])
        bv = cp.tile([P, 1], F32, tag="bv", name="bv"); nc.sync.dma_start(bv[:], t["bv"][:])
        idn = cp.tile([P, P], F32, tag="idn", name="idn"); nc.sync.dma_start(idn[:], t["ident"][:])
        on1 = cp.tile([P, 1], F32, tag="on1", name="on1"); nc.sync.dma_start(on1[:], t["ones128"][:])
        oh = cp.tile([P, E], F32, tag="oh", name="oh"); nc.sync.dma_start(oh[:], t["onehot"][:])
        gb = cp.tile([P, E], F32, tag="gb", name="gb"); nc.sync.dma_start(gb[:], t["gb128"][:])

        # ---- DRAM bounce buffers for collectives ----
        a2a_in = dp.tile([NC * P, 512], F32, tag="a2a_in", name="a2a_in")
        a2a_out = dp.tile([NC * P, 512], F32, tag="a2a_out", name="a2a_out")
        agx_in = dp.tile([D, TOK], BF16, tag="agx_in", name="agx_in")
        agx_out = dp.tile([NC * D, TOK], BF16, tag="agx_out", name="agx_out", addr_space="Shared")
        agl_in = dp.tile([TOK, E], F32, tag="agl_in", name="agl_in")
        agl_out = dp.tile([N, E], F32, tag="agl_out", name="agl_out", addr_space="Shared")
        rs_in0 = dp.tile([N // 2, D], F32, tag="rs_in0", name="rs_in0")
        rs_in1 = dp.tile([N // 2, D], F32, tag="rs_in1", name="rs_in1")
        rs_out0 = dp.tile([TOK // 2, D], F32, tag="rs_out0", name="rs_out0")
        rs_out1 = dp.tile([TOK // 2, D], F32, tag="rs_out1", name="rs_out1")

        # =================== PHASE A: attention ===================
        _sc = _scope(nc, 'attn')
        with (
            tc.tile_pool(name="attn", bufs=1) as ap,
            tc.tile_pool(name="attn2", bufs=2) as ap2,
            tc.tile_pool(name="apsum", bufs=2, space="PSUM") as pqk,
        ):
            for b in range(B):
                qT = ap.tile([P, S], F32R, tag="qT", name="qT")
                kT = ap.tile([P, S], F32R, tag="kT", name="kT")
                vT = ap.tile([P, S], F32, tag="vT", name="vT")
                for c in range(QC):
                    xc = [ap2.tile([P, 512], F32, tag=f"xc{k}", name=f"xc{k}", bufs=2)
                          for k in range(8)]
                    xcr = [ap2.tile([P, 512], F32R, tag=f"xcr{k}", name=f"xcr{k}", bufs=2)
                           for k in range(8)]
                    for k in range(8):
                        nc.sync.dma_start(
                            xc[k][:], xt[b, k * P:(k + 1) * P, c * 512:(c + 1) * 512])
                        nc.vector.tensor_copy(xcr[k][:], xc[k][:])
                    for wtiles, bias, dst in ((wq, bq, qT), (wk, bk, kT), (wv, bv, vT)):
                        ps = pqk.tile([P, 512], F32, tag="qkv_ps", name="qkv_ps")
                        for k in range(8):
                            nc.tensor.matmul(ps[:], wtiles[k][:], xcr[k][:],
                                             start=(k == 0), stop=(k == 7))
                        nc.scalar.activation(dst[:, c * 512:(c + 1) * 512], ps[:],
                                             AF.Identity, bias=bias[:])
                # v natural [tok,128] via PE transpose
                vn = [ap.tile([P, P], F32R, tag=f"vn{i}", name=f"vn{i}") for i in range(16)]
                for i in range(16):
                    pt = pqk.tile([P, P], F32, tag="vt_ps", name="vt_ps", bufs=1)
                    nc.tensor.transpose(pt[:], vT[:, i * P:(i + 1) * P], idn[:])
                    nc.vector.tensor_copy(vn[i][:], pt[:])
                for qc in range(QC):
                    g = b * QC + qc  # global 512-token chunk id
                    est = {}
                    for kt in range(16):
                        for h in range(HPC):
                            hs = slice(h * HD, (h + 1) * HD)
                            ps = pqk.tile([P, 512], F32, tag="s_ps", name="s_ps")
                            nc.tensor.matmul(ps[:], kT[hs, kt * P:(kt + 1) * P],
                                             qT[hs, qc * 512:(qc + 1) * 512],
                                             start=True, stop=True)
                            e = ap2.tile([P, 512], F32R, tag=f"es{h}", name=f"es{h}",
                                         bufs=17)
                            est[(h, kt)] = e
                            nc.scalar.activation(e[:], ps[:], AF.Exp, scale=0.125)
                    for h in range(HPC):
                        pctx = pqk.tile([HD, 512], F32, tag="ctx_ps", name="ctx_ps")
                        pse = pqk.tile([1, 512], F32, tag="se_ps", name="se_ps", bufs=1)
                        for kt in range(16):
                            nc.tensor.matmul(pctx[:], vn[kt][:, h * HD:(h + 1) * HD],
                                             est[(h, kt)][:],
                                             start=(kt == 0), stop=(kt == 15))
                            nc.tensor.matmul(pse[:], on1[:],
                                             est[(h, kt)][:].bitcast(F32),
                                             start=(kt == 0), stop=(kt == 15))
                        rec = ap2.tile([1, 512], F32, tag="rec", name="rec")
                        nc.vector.reciprocal(rec[:], pse[:])
                        bc = ap2.tile([HD, 512], F32, tag="bc", name="bc")
                        nc.gpsimd.partition_broadcast(bc[:], rec[:], channels=HD)
                        cs = ap2.tile([HD, 512], F32, tag="cs", name="cs")
                        nc.vector.tensor_tensor(cs[:], pctx[:], bc[:], op=ALU.mult)
                        nc.sync.dma_start(
                            a2a_in[g * P + h * HD: g * P + (h + 1) * HD, :], cs[:])

        _sc.close()
        _sc = _scope(nc, 'coll_a2a')
        nc.gpsimd.collective_compute(
            "AllToAll", ALU.bypass, ins=[a2a_in.opt()], outs=[a2a_out.opt()],
            replica_groups=[list(range(NC))])

        _sc.close()
        # =================== PHASE B: out-proj + LN1 + logits ===================
        _sc = _scope(nc, 'oproj_ln1')
        x1 = [pp.tile([P, D], F32, tag=f"x1_{tb}", name=f"x1_{tb}") for tb in range(4)]
        with (
            tc.tile_pool(name="bpool", bufs=2) as bp,
            tc.tile_pool(name="bpool1", bufs=1) as bp1,
            tc.tile_pool(name="bpsum", bufs=2, space="PSUM") as bps,
        ):
            x1t = [bp1.tile([P, TOK], F32, tag=f"x1t{dt}", name=f"x1t{dt}") for dt in range(8)]
            ow = [bp1.tile([P, D], F32, tag=f"ow{j}", name=f"ow{j}") for j in range(8)]
            for j in range(8):
                nc.sync.dma_start(ow[j][:], t["owt"][j * P:(j + 1) * P, :])
            gw = [bp1.tile([P, E], F32, tag=f"gw{k}", name=f"gw{k}") for k in range(8)]
            for k in range(8):
                nc.sync.dma_start(gw[k][:], t["gwt"][k * P:(k + 1) * P, :])
            ctxf = [bp1.tile([P, 512], F32, tag=f"cf{j}", name=f"cf{j}") for j in range(8)]
            ctxr = [bp1.tile([P, 512], F32R, tag=f"cr{j}", name=f"cr{j}") for j in range(8)]
            owr = [bp1.tile([P, D], F32R, tag=f"owr{j}", name=f"owr{j}") for j in range(8)]
            for j in range(8):
                nc.sync.dma_start(ctxf[j][:], a2a_out[j * P:(j + 1) * P, :])
                nc.vector.tensor_copy(ctxr[j][:], ctxf[j][:])
                nc.vector.tensor_copy(owr[j][:], ow[j][:])
            obt = bp1.tile([P, D], F32, tag="obt", name="obt")
            nc.sync.dma_start(obt[:], t["ob128"][:])
            g1 = bp1.tile([P, D], F32, tag="g1", name="g1"); nc.sync.dma_start(g1[:], t["lg1"][:])
            b1 = bp1.tile([P, D], F32, tag="b1", name="b1"); nc.sync.dma_start(b1[:], t["lb1"][:])
            for tb in range(4):
                xst = bp.tile([P, D], F32, tag="xst", name="xst")
                nc.sync.dma_start(xst[:], xs[tb * P:(tb + 1) * P, :])
                z = bp.tile([P, D], F32, tag="z", name="z")
                for dc in range(2):
                    ps = bps.tile([P, 512], F32, tag="o_ps", name="o_ps")
                    for j in range(8):
                        nc.tensor.matmul(ps[:], ctxr[j][:, tb * P:(tb + 1) * P],
                                         owr[j][:, dc * 512:(dc + 1) * 512],
                                         start=(j == 0), stop=(j == 7))
                    zc = z[:, dc * 512:(dc + 1) * 512]
                    nc.vector.tensor_tensor(zc, ps[:], xst[:, dc * 512:(dc + 1) * 512],
                                            op=ALU.add)
                    nc.vector.tensor_tensor(zc, zc, obt[:, dc * 512:(dc + 1) * 512],
                                            op=ALU.add)
                _layernorm(nc, bp, z, x1[tb], g1, b1)
                nc.sync.dma_start(t["dbg_x1"][tb * P:(tb + 1) * P, :], x1[tb][:])
                # transpose x1 -> x1t (+bf16 copy for AG)
                for dt in range(8):
                    pt = bps.tile([P, P], F32, tag="t_ps", name="t_ps")
                    nc.tensor.transpose(pt[:], x1[tb][:, dt * P:(dt + 1) * P], idn[:])
                    nc.vector.tensor_copy(x1t[dt][:, tb * P:(tb + 1) * P], pt[:])
                    xb = bp.tile([P, P], BF16, tag="xb", name="xb")
                    nc.vector.tensor_copy(xb[:], pt[:])
                    nc.sync.dma_start(
                        agx_in[dt * P:(dt + 1) * P, tb * P:(tb + 1) * P], xb[:])
                # logits
                pl = bps.tile([P, E], F32, tag="l_ps", name="l_ps")
                for dt in range(8):
                    nc.tensor.matmul(pl[:], x1t[dt][:, tb * P:(tb + 1) * P], gw[dt][:],
                                     start=(dt == 0), stop=(dt == 7))
                lg = bp.tile([P, E], F32, tag="lgt", name="lgt")
                nc.vector.tensor_tensor(lg[:], pl[:], gb[:], op=ALU.add)
                nc.sync.dma_start(agl_in[tb * P:(tb + 1) * P, :], lg[:])
                nc.sync.dma_start(t["dbg_lg"][tb * P:(tb + 1) * P, :], lg[:])

        _sc.close()
        _sc = _scope(nc, 'coll_ag')
        nc.gpsimd.collective_compute(
            "AllGather", ALU.bypass, ins=[agx_in.opt()], outs=[agx_out.opt()],
            replica_groups=[list(range(NC))])
        nc.gpsimd.collective_compute(
            "AllGather", ALU.bypass, ins=[agl_in.opt()], outs=[agl_out.opt()],
            replica_groups=[list(range(NC))])

        _sc.close()
        # =================== PHASE C: routing ===================
        _sc = _scope(nc, 'routing')
        ce_all = pp.tile([P, 32], F32, tag="ce_all", name="ce_all")
        with tc.tile_pool(name="route", bufs=3) as rp:
            uacc = pp.tile([P, E], F32, tag="uacc", name="uacc")
            nc.vector.memset(uacc[:], 0.0)
            for tb in range(32):
                lg = rp.tile([P, E], F32, tag="rlg", name="rlg")
                nc.sync.dma_start(lg[:], agl_out[tb * P:(tb + 1) * P, :])
                m1 = rp.tile([P, 1], F32, tag="m1", name="m1")
                nc.vector.reduce_max(m1[:], lg[:], axis=mybir.AxisListType.X)
                mk1 = rp.tile([P, E], F32, tag="mk1", name="mk1")
                nc.vector.tensor_scalar(mk1[:], lg[:], m1[:], None, op0=ALU.is_equal)
                msk = rp.tile([P, E], F32, tag="msk", name="msk")
                nc.vector.tensor_scalar(msk[:], mk1[:], -1e9, None, op0=ALU.mult)
                nc.vector.tensor_tensor(msk[:], msk[:], lg[:], op=ALU.add)
                m2 = rp.tile([P, 1], F32, tag="m2", name="m2")
                nc.vector.reduce_max(m2[:], msk[:], axis=mybir.AxisListType.X)
                mk2 = rp.tile([P, E], F32, tag="mk2", name="mk2")
                nc.vector.tensor_scalar(mk2[:], msk[:], m2[:], None, op0=ALU.is_equal)
                dl = rp.tile([P, 1], F32, tag="dl", name="dl")
                nc.vector.tensor_tensor(dl[:], m2[:], m1[:], op=ALU.subtract)
                ed = rp.tile([P, 1], F32, tag="ed", name="ed")
                nc.scalar.activation(ed[:], dl[:], AF.Exp)
                wA = rp.tile([P, 1], F32, tag="wA", name="wA")
                nc.vector.tensor_scalar(wA[:], ed[:], 1.0, None, op0=ALU.add)
                nc.vector.reciprocal(wA[:], wA[:])
                wB = rp.tile([P, 1], F32, tag="wB", name="wB")
                nc.vector.tensor_scalar(wB[:], wA[:], -1.0, 1.0, op0=ALU.mult,
                                        op1=ALU.add)
                cmb = rp.tile([P, E], F32, tag="cmb", name="cmb")
                nc.vector.tensor_scalar(cmb[:], mk1[:], wA[:], None, op0=ALU.mult)
                cb2 = rp.tile([P, E], F32, tag="cb2", name="cb2")
                nc.vector.tensor_scalar(cb2[:], mk2[:], wB[:], None, op0=ALU.mult)
                nc.vector.tensor_tensor(cmb[:], cmb[:], cb2[:], op=ALU.add)
                nc.vector.tensor_tensor(cb2[:], cmb[:], oh[:], op=ALU.mult)
                nc.vector.reduce_sum(ce_all[:, tb:tb + 1], cb2[:],
                                     axis=mybir.AxisListType.X)
                # gates for lb_loss
                ge = rp.tile([P, E], F32, tag="ge", name="ge")
                nc.vector.tensor_scalar(ge[:], lg[:], m1[:], None, op0=ALU.subtract)
                nc.scalar.activation(ge[:], ge[:], AF.Exp)
                gs = rp.tile([P, 1], F32, tag="gs", name="gs")
                nc.vector.reduce_sum(gs[:], ge[:], axis=mybir.AxisListType.X)
                nc.vector.reciprocal(gs[:], gs[:])
                nc.vector.tensor_scalar(ge[:], ge[:], gs[:], None, op0=ALU.mult)
                nc.vector.tensor_tensor(uacc[:], uacc[:], ge[:], op=ALU.add)
            ua = rp.tile([P, E], F32, tag="ua", name="ua")
            nc.gpsimd.partition_all_reduce(ua[:], uacc[:], channels=P,
                                           reduce_op=bass.bass_isa.ReduceOp.add)
            us = rp.tile([1, E], F32, tag="us", name="us")
            nc.vector.tensor_scalar(us[:], ua[0:1, :], 1.0 / N, None, op0=ALU.mult)
            nc.scalar.activation(us[:], us[:], AF.Square)
            lbv = rp.tile([1, 1], F32, tag="lbv", name="lbv")
            nc.vector.reduce_sum(lbv[:], us[:], axis=mybir.AxisListType.X)
            nc.vector.tensor_scalar(lbv[:], lbv[:], float(E), None, op0=ALU.mult)
            nc.sync.dma_start(t["lbo"][:], lbv[:])

        _sc.close()
        # =================== PHASE D: FFN ===================
        _sc = _scope(nc, 'ffn')
        with (
            tc.tile_pool(name="wpool", bufs=1) as wp,
            tc.tile_pool(name="fpool", bufs=1) as fp,
            tc.tile_pool(name="fpool2", bufs=3) as fp2,
            tc.tile_pool(name="fpsum", bufs=2, space="PSUM") as fps,
        ):
            w1 = [wp.tile([P, FF], BF16, tag=f"w1_{dt}", name=f"w1_{dt}") for dt in range(8)]
            for dt in range(8):
                nc.sync.dma_start(w1[dt][:], t["w1t"][dt * P:(dt + 1) * P, :])
            w2 = [wp.tile([P, D], BF16, tag=f"w2_{ft}", name=f"w2_{ft}") for ft in range(32)]
            for ft in range(32):
                nc.sync.dma_start(w2[ft][:], t["w2t"][ft * P:(ft + 1) * P, :])
            b1t = wp.tile([P, FF // P], F32, tag="b1t", name="b1t")
            nc.sync.dma_start(b1t[:], t["b1e"][:])
            b2t = wp.tile([P, D], F32, tag="b2t", name="b2t")
            nc.sync.dma_start(b2t[:], t["b2e128"][:])
            ht = [fp.tile([P, 512], BF16, tag=f"ht{ft}", name=f"ht{ft}") for ft in range(32)]
            for h in range(2):
                rs_in_h = rs_in0 if h == 0 else rs_in1
                for j in range(4):
                    x1c = [fp2.tile([P, 512], BF16, tag=f"x1c{dt}", name=f"x1c{dt}", bufs=1) for dt in range(8)]
                    for dt in range(8):
                        for hf in range(2):
                            r = 2 * j + hf
                            nc.sync.dma_start(
                                x1c[dt][:, hf * 256:(hf + 1) * 256],
                                agx_out[r * D + dt * P: r * D + (dt + 1) * P,
                                        h * 256:(h + 1) * 256])
                    for ft in range(32):
                        ps = fps.tile([P, 512], F32, tag="h_ps", name="h_ps", bufs=4)
                        for dt in range(8):
                            nc.tensor.matmul(ps[:], w1[dt][:, ft * P:(ft + 1) * P],
                                             x1c[dt][:], start=(dt == 0), stop=(dt == 7))
                        nc.scalar.activation(ht[ft][:], ps[:], AF.Relu,
                                             bias=b1t[:, ft:ft + 1])
                    for tb in range(4):
                        r = 2 * j + tb // 2
                        gtb = r * 4 + h * 2 + (tb % 2)
                        for dc in range(2):
                            ps = fps.tile([P, 512], F32, tag="y_ps", name="y_ps", bufs=4)
                            for ft in range(32):
                                nc.tensor.matmul(ps[:], ht[ft][:, tb * P:(tb + 1) * P],
                                                 w2[ft][:, dc * 512:(dc + 1) * 512],
                                                 start=(ft == 0), stop=(ft == 31))
                            yw = fp2.tile([P, 512], F32, tag="yw", name="yw", bufs=2)
                            nc.vector.tensor_tensor(yw[:], ps[:],
                                                    b2t[:, dc * 512:(dc + 1) * 512],
                                                    op=ALU.add)
                            nc.vector.tensor_scalar(yw[:], yw[:],
                                                    ce_all[:, gtb:gtb + 1], None,
                                                    op0=ALU.mult)
                            row = r * 256 + (tb % 2) * P
                            nc.sync.dma_start(
                                rs_in_h[row:row + P, dc * 512:(dc + 1) * 512],
                                yw[:])
                nc.gpsimd.collective_compute(
                    "ReduceScatter", ALU.add, ins=[rs_in_h.opt()],
                    outs=[(rs_out0 if h == 0 else rs_out1).opt()],
                    replica_groups=[list(range(NC))])

        _sc.close()
        # =================== PHASE E: residual + LN2 ===================
        _sc = _scope(nc, 'ln2')
        with tc.tile_pool(name="epool", bufs=2) as ep:
            g2 = ep.tile([P, D], F32, tag="g2", name="g2"); nc.sync.dma_start(g2[:], t["lg2"][:])
            b2 = ep.tile([P, D], F32, tag="b2", name="b2"); nc.sync.dma_start(b2[:], t["lb2"][:])
            for tb in range(4):
                hh, ii = tb // 2, tb % 2
                rs_out_h = rs_out0 if hh == 0 else rs_out1
                ys = ep.tile([P, D], F32, tag="ys", name="ys")
                nc.sync.dma_start(ys[:], rs_out_h[ii * P:(ii + 1) * P, :])
                z2 = ep.tile([P, D], F32, tag="z2", name="z2")
                nc.vector.tensor_tensor(z2[:], ys[:], x1[tb][:], op=ALU.add)
                x2 = ep.tile([P, D], F32, tag="x2", name="x2")
                _layernorm(nc, ep, z2, x2, g2, b2)
                nc.sync.dma_start(t["x2s"][tb * P:(tb + 1) * P, :], x2[:])
        _sc.close()


def _layernorm(nc, pool, z, out, g, b):
    """out = (z - mean)/sqrt(var+eps) * g + b along free dim (D)."""
    mean = pool.tile([P, 1], F32, tag="ln_m", name="ln_m")
    nc.vector.reduce_sum(mean[:], z[:], axis=mybir.AxisListType.X)
    nc.vector.tensor_scalar(mean[:], mean[:], 1.0 / D, None, op0=ALU.mult)
    zc = pool.tile([P, D], F32, tag="ln_zc", name="ln_zc")
    nc.vector.tensor_scalar(zc[:], z[:], mean[:], None, op0=ALU.subtract)
    sq = pool.tile([P, D], F32, tag="ln_sq", name="ln_sq")
    nc.vector.tensor_tensor(sq[:], zc[:], zc[:], op=ALU.mult)
    var = pool.tile([P, 1], F32, tag="ln_v", name="ln_v")
    nc.vector.reduce_sum(var[:], sq[:], axis=mybir.AxisListType.X)
    nc.vector.tensor_scalar(var[:], var[:], 1.0 / D, EPS, op0=ALU.mult, op1=ALU.add)
    nc.scalar.activation(var[:], var[:], AF.Sqrt)
    nc.vector.reciprocal(var[:], var[:])
    nc.vector.tensor_scalar(zc[:], zc[:], var[:], None, op0=ALU.mult)
    nc.vector.tensor_tensor(zc[:], zc[:], g[:], op=ALU.mult)
    nc.vector.tensor_tensor(out[:], zc[:], b[:], op=ALU.add)


def make_inputs(inputs):
    """Build per-core in_maps from full inputs."""
    x = np.asarray(inputs["x"], np.float32)
    ipw = np.asarray(inputs["in_proj_w"], np.float32)
    ipb = np.asarray(inputs["in_proj_b"], np.float32)
    out_w = np.asarray(inputs["out_w"], np.float32)
    out_b = np.asarray(inputs["out_b"], np.float32)
    ln1_g = np.asarray(inputs["ln1_g"], np.float32)
    ln1_b = np.asarray(inputs["ln1_b"], np.float32)
    gate_w = np.asarray(inputs["gate_w"], np.float32)
    gate_b = np.asarray(inputs["gate_b"], np.float32)
    w1 = np.asarray(inputs["w1"], np.float32)
    b1 = np.asarray(inputs["b1"], np.float32)
    w2 = np.asarray(inputs["w2"], np.float32)
    b2 = np.asarray(inputs["b2"], np.float32)
    ln2_g = np.asarray(inputs["ln2_g"], np.float32)
    ln2_b = np.asarray(inputs["ln2_b"], np.float32)

    xt = np.ascontiguousarray(np.transpose(x, (0, 2, 1)))  # [B, D, S]
    xf = x.reshape(N, D)
    bc = lambda v: np.ascontiguousarray(np.broadcast_to(v, (P, v.shape[0]))).astype(np.float32)
    common = {
        "xt": xt, "owt": np.ascontiguousarray(out_w.T), "ob128": bc(out_b),
        "lg1": bc(ln1_g), "lb1": bc(ln1_b), "lg2": bc(ln2_g), "lb2": bc(ln2_b),
        "gwt": np.ascontiguousarray(gate_w.T), "gb128": bc(gate_b),
        "ident": np.eye(P, dtype=np.float32),
        "ones128": np.ones((P, 1), np.float32),
    }
    maps = []
    for c in range(NC):
        r = slice(P * c, P * (c + 1))
        oh = np.zeros((P, E), np.float32); oh[:, c] = 1.0
        m = dict(common)
        m.update({
            "xs": xf[TOK * c: TOK * (c + 1)],
            "wqt": np.ascontiguousarray(ipw[r].T),
            "wkt": np.ascontiguousarray(ipw[D:][r].T),
            "wvt": np.ascontiguousarray(ipw[2 * D:][r].T),
            "bq": ipb[r][:, None].copy(), "bk": ipb[D:][r][:, None].copy(),
            "bv": ipb[2 * D:][r][:, None].copy(),
            "w1t": np.ascontiguousarray(w1[c].T).astype(ml_dtypes.bfloat16),
            "b1e": np.ascontiguousarray(b1[c].reshape(FF // P, P).T),
            "w2t": np.ascontiguousarray(w2[c].T).astype(ml_dtypes.bfloat16),
            "b2e128": bc(b2[c]),
            "onehot": oh,
        })
        maps.append(m)
    return maps


def run(inputs, trace=False):
    if "nc" not in _CACHE:
        _CACHE["nc"] = build_nc()
    nc = _CACHE["nc"]
    maps = make_inputs(inputs)
    res = run_bass_kernel_spmd(nc, maps, core_ids=list(range(NC)), trace=trace)
    x2 = np.concatenate([res.results[c]["x2s"] for c in range(NC)], axis=0)
    lb = np.float32(res.results[0]["lb"][0, 0])
    return (x2.reshape(B, S, D), lb), res


def kernel(**inputs):
    out, _ = run(inputs, trace=False)
    return out


# revision 22
# speedup vs baseline: 1.0633x; 1.0633x over previous
"""MoE Transformer Block kernel for Trainium2, 8 NeuronCores.

Sharding: attention head-parallel (2 heads/core), MoE expert-parallel
(1 expert/core, dense-expert V1), token-sliced LN/output (512 tok/core).
Collectives: AllToAll(ctx) -> AllGather(x1 bf16) -> AllGather(logits)
-> ReduceScatter(moe out).
"""
import sys, types

# ---- antenv.axon_hooks shim (image's antenv lacks this tiny registry) ----
def _install_hook_shim():
    try:
        import antenv
    except ImportError:
        return
    if "antenv.axon_hooks" in sys.modules:
        return
    m = types.ModuleType("antenv.axon_hooks")
    m._hook = None
    def _set(h): m._hook = h
    def _get(): return m._hook
    m.set_axon_ntff_profile_hook = _set
    m.get_axon_ntff_profile_hook = _get
    sys.modules["antenv.axon_hooks"] = m
    antenv.axon_hooks = m
    try:
        from trn_agent_boot.trn_boot import _ntff_profile_via_ctypes
        import os
        if os.path.exists("/opt/axon/libaxon_pjrt.so"):
            _set(_ntff_profile_via_ctypes("/opt/axon/libaxon_pjrt.so"))
    except Exception:
        pass

_install_hook_shim()

import numpy as np
import ml_dtypes
import concourse.bass as bass
import concourse.bacc as bacc
import concourse.tile as tile
from concourse import mybir
from concourse.bass_utils import run_bass_kernel_spmd

F32 = mybir.dt.float32
F32R = mybir.dt.float32r
BF16 = mybir.dt.bfloat16
AF = mybir.ActivationFunctionType
ALU = mybir.AluOpType

B, S, D, H, FF, E, TOPK = 2, 2048, 1024, 16, 4096, 8, 2
N = B * S          # 4096 tokens
NC = 8             # cores
TOK = N // NC      # 512 tokens per core
HPC = H // NC      # 2 heads per core
HD = D // H        # 64
EPS = 1e-5
P = 128

_CACHE = {}


def build_nc():
    nc = bacc.Bacc("TRN2", target_bir_lowering=False, debug=False, num_devices=NC)

    # ---------------- DRAM I/O ----------------
    xt = nc.dram_tensor("xt", [B, D, S], F32, kind="ExternalInput")       # x[b].T
    xs = nc.dram_tensor("xs", [TOK, D], F32, kind="ExternalInput")        # x slice
    wqt = nc.dram_tensor("wqt", [D, P], F32, kind="ExternalInput")
    wkt = nc.dram_tensor("wkt", [D, P], F32, kind="ExternalInput")
    wvt = nc.dram_tensor("wvt", [D, P], F32, kind="ExternalInput")
    bq = nc.dram_tensor("bq", [P, 1], F32, kind="ExternalInput")
    bk = nc.dram_tensor("bk", [P, 1], F32, kind="ExternalInput")
    bv = nc.dram_tensor("bv", [P, 1], F32, kind="ExternalInput")
    owt = nc.dram_tensor("owt", [D, D], F32, kind="ExternalInput")        # out_w.T
    ob128 = nc.dram_tensor("ob128", [P, D], F32, kind="ExternalInput")
    lg1 = nc.dram_tensor("lg1", [P, D], F32, kind="ExternalInput")
    lb1 = nc.dram_tensor("lb1", [P, D], F32, kind="ExternalInput")
    lg2 = nc.dram_tensor("lg2", [P, D], F32, kind="ExternalInput")
    lb2 = nc.dram_tensor("lb2", [P, D], F32, kind="ExternalInput")
    gwt = nc.dram_tensor("gwt", [D, E], F32, kind="ExternalInput")        # gate_w.T
    gb128 = nc.dram_tensor("gb128", [P, E], F32, kind="ExternalInput")
    w1t = nc.dram_tensor("w1t", [D, FF], BF16, kind="ExternalInput")      # w1[e].T
    b1e = nc.dram_tensor("b1e", [P, FF // P], F32, kind="ExternalInput")  # col ft
    w2t = nc.dram_tensor("w2t", [FF, D], BF16, kind="ExternalInput")      # w2[e].T
    b2e128 = nc.dram_tensor("b2e128", [P, D], F32, kind="ExternalInput")
    onehot = nc.dram_tensor("onehot", [P, E], F32, kind="ExternalInput")
    ident = nc.dram_tensor("ident", [P, P], F32, kind="ExternalInput")
    ones128 = nc.dram_tensor("ones128", [P, 1], F32, kind="ExternalInput")

    x2s = nc.dram_tensor("x2s", [TOK, D], F32, kind="ExternalOutput")
    lbo = nc.dram_tensor("lb", [1, 1], F32, kind="ExternalOutput")
    dbg_x1 = nc.dram_tensor("dbg_x1", [TOK, D], F32, kind="ExternalOutput")
    dbg_lg = nc.dram_tensor("dbg_lg", [TOK, E], F32, kind="ExternalOutput")

    with tile.TileContext(nc) as tc:
        _emit(nc, tc, locals())
    nc.compile()
    return nc



def _scope(nc, name):
    import contextlib
    es = contextlib.ExitStack()
    es.enter_context(nc.named_scope(name))
    return es

def _emit(nc, tc, t):
    xt, xs = t["xt"], t["xs"]
    QC = S // 512  # 4 q-chunks per batch

    with (
        tc.tile_pool(name="const", bufs=1) as cp,
        tc.tile_pool(name="persist", bufs=1) as pp,
        tc.tile_pool(name="dram", bufs=1, space="DRAM") as dp,
    ):
        # ---- constants ----
        wq = [cp.tile([P, P], F32R, tag=f"wq{k}", name=f"wq{k}") for k in range(8)]
        wk = [cp.tile([P, P], F32R, tag=f"wk{k}", name=f"wk{k}") for k in range(8)]
        wv = [cp.tile([P, P], F32R, tag=f"wv{k}", name=f"wv{k}") for k in range(8)]
        with tc.tile_pool(name="wstage", bufs=2) as stp:
            for k in range(8):
                wqf = stp.tile([P, P], F32, tag="wqf", name="wqf")
                wkf = stp.tile([P, P], F32, tag="wkf", name="wkf")
                wvf = stp.tile([P, P], F32, tag="wvf", name="wvf")
                nc.sync.dma_start(wqf[:], t["wqt"][k * P:(k + 1) * P, :])
                nc.sync.dma_start(wkf[:], t["wkt"][k * P:(k + 1) * P, :])
                nc.sync.dma_start(wvf[:], t["wvt"][k * P:(k + 1) * P, :])
                nc.vector.tensor_copy(wq[k][:], wqf[:])
                nc.vector.tensor_copy(wk[k][:], wkf[:])
                nc.vector.tensor_copy(wv[k][:], wvf[:])
        bq = cp.tile([P, 1], F32, tag="bq", name="bq"); nc.sync.dma_start(bq[:], t["bq"][:])
        bk = cp.tile([P, 1], F32, tag="bk", name="bk"); nc.sync.dma_start(bk[:], t["bk"][:])
        bv = cp.tile([P, 1], F32, tag="bv", name="bv"); nc.sync.dma_start(bv[:], t["bv"][:])
        idn = cp.tile([P, P], F32, tag="idn", name="idn"); nc.sync.dma_start(idn[:], t["ident"][:])
        on1 = cp.tile([P, 1], F32, tag="on1", name="on1"); nc.sync.dma_start(on1[:], t["ones128"][:])
        oh = cp.tile([P, E], F32, tag="oh", name="oh"); nc.sync.dma_start(oh[:], t["onehot"][:])
        gb = cp.tile([P, E], F32, tag="gb", name="gb"); nc.sync.dma_start(gb[:], t["gb128"][:])

        # ---- DRAM bounce buffers for collectives ----
        a2a_in = dp.tile([NC * P, 512], F32, tag="a2a_in", name="a2a_in")
        a2a_out = dp.tile([NC * P, 512], F32, tag="a2a_out", name="a2a_out")
        agx_in = dp.tile([D, TOK], BF16, tag="agx_in", name="agx_in")
        agx_out = dp.tile([NC * D, TOK], BF16, tag="agx_out", name="agx_out", addr_space="Shared")
        agl_in = dp.tile([TOK, E], F32, tag="agl_in", name="agl_in")
        agl_out = dp.tile([N, E], F32, tag="agl_out", name="agl_out", addr_space="Shared")
        rs_in0 = dp.tile([N // 2, D], F32, tag="rs_in0", name="rs_in0")
        rs_in1 = dp.tile([N // 2, D], F32, tag="rs_in1", name="rs_in1")
        rs_out0 = dp.tile([TOK // 2, D], F32, tag="rs_out0", name="rs_out0")
        rs_out1 = dp.tile([TOK // 2, D], F32, tag="rs_out1", name="rs_out1")

        # =================== PHASE A: attention ===================
        _sc = _scope(nc, 'attn')
        with (
            tc.tile_pool(name="attn", bufs=1) as ap,
            tc.tile_pool(name="attn2", bufs=2) as ap2,
            tc.tile_pool(name="apsum", bufs=2, space="PSUM") as pqk,
        ):
            for b in range(B):
                qT = ap.tile([P, S], F32R, tag="qT", name="qT")
                kT = ap.tile([P, S], F32R, tag="kT", name="kT")
                vT = ap.tile([P, S], F32, tag="vT", name="vT")
                for c in range(QC):
                    xc = [ap2.tile([P, 512], F32, tag=f"xc{k}", name=f"xc{k}", bufs=2)
                          for k in range(8)]
                    xcr = [ap2.tile([P, 512], F32R, tag=f"xcr{k}", name=f"xcr{k}", bufs=2)
                           for k in range(8)]
                    for k in range(8):
                        nc.sync.dma_start(
                            xc[k][:], xt[b, k * P:(k + 1) * P, c * 512:(c + 1) * 512])
                        nc.vector.tensor_copy(xcr[k][:], xc[k][:])
                    for wtiles, bias, dst in ((wq, bq, qT), (wk, bk, kT), (wv, bv, vT)):
                        ps = pqk.tile([P, 512], F32, tag="qkv_ps", name="qkv_ps")
                        for k in range(8):
                            nc.tensor.matmul(ps[:], wtiles[k][:], xcr[k][:],
                                             start=(k == 0), stop=(k == 7))
                        nc.scalar.activation(dst[:, c * 512:(c + 1) * 512], ps[:],
                                             AF.Identity, bias=bias[:])
                # v natural [tok,128] via PE transpose
                vn = [ap.tile([P, 130], F32R, tag=f"vn{i}", name=f"vn{i}") for i in range(16)]
                for i in range(16):
                    pt = pqk.tile([P, P], F32, tag="vt_ps", name="vt_ps", bufs=1)
                    nc.tensor.transpose(pt[:], vT[:, i * P:(i + 1) * P], idn[:])
                    nc.vector.tensor_copy(vn[i][:, 0:HD], pt[:, 0:HD])
                    nc.vector.tensor_copy(vn[i][:, 65:65 + HD], pt[:, HD:2 * HD])
                    nc.vector.tensor_copy(vn[i][:, HD:HD + 1], on1[:])
                    nc.vector.tensor_copy(vn[i][:, 129:130], on1[:])
                for qc in range(QC):
                    g = b * QC + qc  # global 512-token chunk id
                    est = {}
                    for kt in range(16):
                        for h in range(HPC):
                            hs = slice(h * HD, (h + 1) * HD)
                            ps = pqk.tile([P, 512], F32, tag="s_ps", name="s_ps")
                            nc.tensor.matmul(ps[:], kT[hs, kt * P:(kt + 1) * P],
                                             qT[hs, qc * 512:(qc + 1) * 512],
                                             start=True, stop=True)
                            e = ap2.tile([P, 512], F32R, tag=f"es{h}", name=f"es{h}",
                                         bufs=17)
                            est[(h, kt)] = e
                            nc.scalar.activation(e[:], ps[:], AF.Exp, scale=0.125)
                    for h in range(HPC):
                        pctx = pqk.tile([HD + 1, 512], F32, tag="ctx_ps", name="ctx_ps")
                        for kt in range(16):
                            nc.tensor.matmul(pctx[:], vn[kt][:, h * 65:h * 65 + 65],
                                             est[(h, kt)][:],
                                             start=(kt == 0), stop=(kt == 15))
                        rec = ap2.tile([1, 512], F32, tag="rec", name="rec")
                        nc.vector.reciprocal(rec[:], pctx[HD:HD + 1, :])
                        bc = ap2.tile([HD, 512], F32, tag="bc", name="bc")
                        nc.gpsimd.partition_broadcast(bc[:], rec[:], channels=HD)
                        cs = ap2.tile([HD, 512], F32, tag="cs", name="cs")
                        nc.vector.tensor_tensor(cs[:], pctx[0:HD, :], bc[:], op=ALU.mult)
                        nc.sync.dma_start(
                            a2a_in[g * P + h * HD: g * P + (h + 1) * HD, :], cs[:])

        _sc.close()
        _sc = _scope(nc, 'coll_a2a')
        nc.gpsimd.collective_compute(
            "AllToAll", ALU.bypass, ins=[a2a_in.opt()], outs=[a2a_out.opt()],
            replica_groups=[list(range(NC))])

        _sc.close()
        # =================== PHASE B: out-proj + LN1 + logits ===================
        _sc = _scope(nc, 'oproj_ln1')
        x1 = [pp.tile([P, D], F32, tag=f"x1_{tb}", name=f"x1_{tb}") for tb in range(4)]
        with (
            tc.tile_pool(name="bpool", bufs=2) as bp,
            tc.tile_pool(name="bpool1", bufs=1) as bp1,
            tc.tile_pool(name="bpsum", bufs=2, space="PSUM") as bps,
        ):
            x1t = [bp1.tile([P, TOK], F32, tag=f"x1t{dt}", name=f"x1t{dt}") for dt in range(8)]
            ow = [bp1.tile([P, D], F32, tag=f"ow{j}", name=f"ow{j}") for j in range(8)]
            for j in range(8):
                nc.sync.dma_start(ow[j][:], t["owt"][j * P:(j + 1) * P, :])
            gw = [bp1.tile([P, E], F32, tag=f"gw{k}", name=f"gw{k}") for k in range(8)]
            for k in range(8):
                nc.sync.dma_start(gw[k][:], t["gwt"][k * P:(k + 1) * P, :])
            ctxf = [bp1.tile([P, 512], F32, tag=f"cf{j}", name=f"cf{j}") for j in range(8)]
            ctxr = [bp1.tile([P, 512], F32R, tag=f"cr{j}", name=f"cr{j}") for j in range(8)]
            owr = [bp1.tile([P, D], F32R, tag=f"owr{j}", name=f"owr{j}") for j in range(8)]
            for j in range(8):
                nc.sync.dma_start(ctxf[j][:], a2a_out[j * P:(j + 1) * P, :])
                nc.vector.tensor_copy(ctxr[j][:], ctxf[j][:])
                nc.vector.tensor_copy(owr[j][:], ow[j][:])
            obt = bp1.tile([P, D], F32, tag="obt", name="obt")
            nc.sync.dma_start(obt[:], t["ob128"][:])
            g1 = bp1.tile([P, D], F32, tag="g1", name="g1"); nc.sync.dma_start(g1[:], t["lg1"][:])
            b1 = bp1.tile([P, D], F32, tag="b1", name="b1"); nc.sync.dma_start(b1[:], t["lb1"][:])
            for tb in range(4):
                xst = bp.tile([P, D], F32, tag="xst", name="xst")
                nc.sync.dma_start(xst[:], xs[tb * P:(tb + 1) * P, :])
                z = bp.tile([P, D], F32, tag="z", name="z")
                for dc in range(2):
                    ps = bps.tile([P, 512], F32, tag="o_ps", name="o_ps")
                    for j in range(8):
                        nc.tensor.matmul(ps[:], ctxr[j][:, tb * P:(tb + 1) * P],
                                         owr[j][:, dc * 512:(dc + 1) * 512],
                                         start=(j == 0), stop=(j == 7))
                    zc = z[:, dc * 512:(dc + 1) * 512]
                    nc.vector.tensor_tensor(zc, ps[:], xst[:, dc * 512:(dc + 1) * 512],
                                            op=ALU.add)
                    nc.vector.tensor_tensor(zc, zc, obt[:, dc * 512:(dc + 1) * 512],
                                            op=ALU.add)
                _layernorm(nc, bp, z, x1[tb], g1, b1)
                nc.sync.dma_start(t["dbg_x1"][tb * P:(tb + 1) * P, :], x1[tb][:])
                # transpose x1 -> x1t (+bf16 copy for AG)
                for dt in range(8):
                    pt = bps.tile([P, P], F32, tag="t_ps", name="t_ps")
                    nc.tensor.transpose(pt[:], x1[tb][:, dt * P:(dt + 1) * P], idn[:])
                    nc.vector.tensor_copy(x1t[dt][:, tb * P:(tb + 1) * P], pt[:])
                    xb = bp.tile([P, P], BF16, tag="xb", name="xb")
                    nc.vector.tensor_copy(xb[:], pt[:])
                    nc.sync.dma_start(
                        agx_in[dt * P:(dt + 1) * P, tb * P:(tb + 1) * P], xb[:])
                # logits
                pl = bps.tile([P, E], F32, tag="l_ps", name="l_ps")
                for dt in range(8):
                    nc.tensor.matmul(pl[:], x1t[dt][:, tb * P:(tb + 1) * P], gw[dt][:],
                                     start=(dt == 0), stop=(dt == 7))
                lg = bp.tile([P, E], F32, tag="lgt", name="lgt")
                nc.vector.tensor_tensor(lg[:], pl[:], gb[:], op=ALU.add)
                nc.sync.dma_start(agl_in[tb * P:(tb + 1) * P, :], lg[:])
                nc.sync.dma_start(t["dbg_lg"][tb * P:(tb + 1) * P, :], lg[:])

        _sc.close()
        _sc = _scope(nc, 'coll_ag')
        nc.gpsimd.collective_compute(
            "AllGather", ALU.bypass, ins=[agx_in.opt()], outs=[agx_out.opt()],
            replica_groups=[list(range(NC))])
        nc.gpsimd.collective_compute(
            "AllGather", ALU.bypass, ins=[agl_in.opt()], outs=[agl_out.opt()],
            replica_groups=[list(range(NC))])

        _sc.close()
        # =================== PHASE C: routing ===================
        _sc = _scope(nc, 'routing')
        ce_all = pp.tile([P, 32], F32, tag="ce_all", name="ce_all")
        with tc.tile_pool(name="route", bufs=3) as rp:
            uacc = pp.tile([P, E], F32, tag="uacc", name="uacc")
            nc.vector.memset(uacc[:], 0.0)
            for tb in range(32):
                lg = rp.tile([P, E], F32, tag="rlg", name="rlg")
                nc.sync.dma_start(lg[:], agl_out[tb * P:(tb + 1) * P, :])
                m1 = rp.tile([P, 1], F32, tag="m1", name="m1")
                nc.vector.reduce_max(m1[:], lg[:], axis=mybir.AxisListType.X)
                mk1 = rp.tile([P, E], F32, tag="mk1", name="mk1")
                nc.vector.tensor_scalar(mk1[:], lg[:], m1[:], None, op0=ALU.is_equal)
                msk = rp.tile([P, E], F32, tag="msk", name="msk")
                nc.vector.tensor_scalar(msk[:], mk1[:], -1e9, None, op0=ALU.mult)
                nc.vector.tensor_tensor(msk[:], msk[:], lg[:], op=ALU.add)
                m2 = rp.tile([P, 1], F32, tag="m2", name="m2")
                nc.vector.reduce_max(m2[:], msk[:], axis=mybir.AxisListType.X)
                mk2 = rp.tile([P, E], F32, tag="mk2", name="mk2")
                nc.vector.tensor_scalar(mk2[:], msk[:], m2[:], None, op0=ALU.is_equal)
                dl = rp.tile([P, 1], F32, tag="dl", name="dl")
                nc.vector.tensor_tensor(dl[:], m2[:], m1[:], op=ALU.subtract)
                ed = rp.tile([P, 1], F32, tag="ed", name="ed")
                nc.scalar.activation(ed[:], dl[:], AF.Exp)
                wA = rp.tile([P, 1], F32, tag="wA", name="wA")
                nc.vector.tensor_scalar(wA[:], ed[:], 1.0, None, op0=ALU.add)
                nc.vector.reciprocal(wA[:], wA[:])
                wB = rp.tile([P, 1], F32, tag="wB", name="wB")
                nc.vector.tensor_scalar(wB[:], wA[:], -1.0, 1.0, op0=ALU.mult,
                                        op1=ALU.add)
                cmb = rp.tile([P, E], F32, tag="cmb", name="cmb")
                nc.vector.tensor_scalar(cmb[:], mk1[:], wA[:], None, op0=ALU.mult)
                cb2 = rp.tile([P, E], F32, tag="cb2", name="cb2")
                nc.vector.tensor_scalar(cb2[:], mk2[:], wB[:], None, op0=ALU.mult)
                nc.vector.tensor_tensor(cmb[:], cmb[:], cb2[:], op=ALU.add)
                nc.vector.tensor_tensor(cb2[:], cmb[:], oh[:], op=ALU.mult)
                nc.vector.reduce_sum(ce_all[:, tb:tb + 1], cb2[:],
                                     axis=mybir.AxisListType.X)
                # gates for lb_loss
                ge = rp.tile([P, E], F32, tag="ge", name="ge")
                nc.vector.tensor_scalar(ge[:], lg[:], m1[:], None, op0=ALU.subtract)
                nc.scalar.activation(ge[:], ge[:], AF.Exp)
                gs = rp.tile([P, 1], F32, tag="gs", name="gs")
                nc.vector.reduce_sum(gs[:], ge[:], axis=mybir.AxisListType.X)
                nc.vector.reciprocal(gs[:], gs[:])
                nc.vector.tensor_scalar(ge[:], ge[:], gs[:], None, op0=ALU.mult)
                nc.vector.tensor_tensor(uacc[:], uacc[:], ge[:], op=ALU.add)
            ua = rp.tile([P, E], F32, tag="ua", name="ua")
            nc.gpsimd.partition_all_reduce(ua[:], uacc[:], channels=P,
                                           reduce_op=bass.bass_isa.ReduceOp.add)
            us = rp.tile([1, E], F32, tag="us", name="us")
            nc.vector.tensor_scalar(us[:], ua[0:1, :], 1.0 / N, None, op0=ALU.mult)
            nc.scalar.activation(us[:], us[:], AF.Square)
            lbv = rp.tile([1, 1], F32, tag="lbv", name="lbv")
            nc.vector.reduce_sum(lbv[:], us[:], axis=mybir.AxisListType.X)
            nc.vector.tensor_scalar(lbv[:], lbv[:], float(E), None, op0=ALU.mult)
            nc.sync.dma_start(t["lbo"][:], lbv[:])

        _sc.close()
        # =================== PHASE D: FFN ===================
        _sc = _scope(nc, 'ffn')
        with (
            tc.tile_pool(name="wpool", bufs=1) as wp,
            tc.tile_pool(name="fpool", bufs=1) as fp,
            tc.tile_pool(name="fpool2", bufs=3) as fp2,
            tc.tile_pool(name="fpsum", bufs=2, space="PSUM") as fps,
        ):
            w1 = [wp.tile([P, FF], BF16, tag=f"w1_{dt}", name=f"w1_{dt}") for dt in range(8)]
            for dt in range(8):
                nc.sync.dma_start(w1[dt][:], t["w1t"][dt * P:(dt + 1) * P, :])
            w2 = [wp.tile([P, D], BF16, tag=f"w2_{ft}", name=f"w2_{ft}") for ft in range(32)]
            for ft in range(32):
                nc.sync.dma_start(w2[ft][:], t["w2t"][ft * P:(ft + 1) * P, :])
            b1t = wp.tile([P, FF // P], F32, tag="b1t", name="b1t")
            nc.sync.dma_start(b1t[:], t["b1e"][:])
            b2t = wp.tile([P, D], F32, tag="b2t", name="b2t")
            nc.sync.dma_start(b2t[:], t["b2e128"][:])
            ht = [fp.tile([P, 512], BF16, tag=f"ht{ft}", name=f"ht{ft}") for ft in range(32)]
            for h in range(2):
                rs_in_h = rs_in0 if h == 0 else rs_in1
                for j in range(4):
                    x1c = [fp2.tile([P, 512], BF16, tag=f"x1c{dt}", name=f"x1c{dt}", bufs=1) for dt in range(8)]
                    for dt in range(8):
                        for hf in range(2):
                            r = 2 * j + hf
                            nc.sync.dma_start(
                                x1c[dt][:, hf * 256:(hf + 1) * 256],
                                agx_out[r * D + dt * P: r * D + (dt + 1) * P,
                                        h * 256:(h + 1) * 256])
                    for ft in range(32):
                        ps = fps.tile([P, 512], F32, tag="h_ps", name="h_ps", bufs=4)
                        for dt in range(8):
                            nc.tensor.matmul(ps[:], w1[dt][:, ft * P:(ft + 1) * P],
                                             x1c[dt][:], start=(dt == 0), stop=(dt == 7))
                        nc.scalar.activation(ht[ft][:], ps[:], AF.Relu,
                                             bias=b1t[:, ft:ft + 1])
                    for tb in range(4):
                        r = 2 * j + tb // 2
                        gtb = r * 4 + h * 2 + (tb % 2)
                        for dc in range(2):
                            ps = fps.tile([P, 512], F32, tag="y_ps", name="y_ps", bufs=4)
                            for ft in range(32):
                                nc.tensor.matmul(ps[:], ht[ft][:, tb * P:(tb + 1) * P],
                                                 w2[ft][:, dc * 512:(dc + 1) * 512],
                                                 start=(ft == 0), stop=(ft == 31))
                            yw = fp2.tile([P, 512], F32, tag="yw", name="yw", bufs=2)
                            nc.vector.tensor_tensor(yw[:], ps[:],
                                                    b2t[:, dc * 512:(dc + 1) * 512],
                                                    op=ALU.add)
                            nc.vector.tensor_scalar(yw[:], yw[:],
                                                    ce_all[:, gtb:gtb + 1], None,
                                                    op0=ALU.mult)
                            row = r * 256 + (tb % 2) * P
                            nc.sync.dma_start(
                                rs_in_h[row:row + P, dc * 512:(dc + 1) * 512],
                                yw[:])
                nc.gpsimd.collective_compute(
                    "ReduceScatter", ALU.add, ins=[rs_in_h.opt()],
                    outs=[(rs_out0 if h == 0 else rs_out1).opt()],
                    replica_groups=[list(range(NC))])

        _sc.close()
        # =================== PHASE E: residual + LN2 ===================
        _sc = _scope(nc, 'ln2')
        with tc.tile_pool(name="epool", bufs=2) as ep:
            g2 = ep.tile([P, D], F32, tag="g2", name="g2"); nc.sync.dma_start(g2[:], t["lg2"][:])
            b2 = ep.tile([P, D], F32, tag="b2", name="b2"); nc.sync.dma_start(b2[:], t["lb2"][:])
            for tb in range(4):
                hh, ii = tb // 2, tb % 2
                rs_out_h = rs_out0 if hh == 0 else rs_out1
                ys = ep.tile([P, D], F32, tag="ys", name="ys")
                nc.sync.dma_start(ys[:], rs_out_h[ii * P:(ii + 1) * P, :])
                z2 = ep.tile([P, D], F32, tag="z2", name="z2")
                nc.vector.tensor_tensor(z2[:], ys[:], x1[tb][:], op=ALU.add)
                x2 = ep.tile([P, D], F32, tag="x2", name="x2")
                _layernorm(nc, ep, z2, x2, g2, b2)
                nc.sync.dma_start(t["x2s"][tb * P:(tb + 1) * P, :], x2[:])
        _sc.close()


def _layernorm(nc, pool, z, out, g, b):
    """out = (z - mean)/sqrt(var+eps) * g + b along free dim (D)."""
    mean = pool.tile([P, 1], F32, tag="ln_m", name="ln_m")
    nc.vector.reduce_sum(mean[:], z[:], axis=mybir.AxisListType.X)
    nc.vector.tensor_scalar(mean[:], mean[:], 1.0 / D, None, op0=ALU.mult)
    zc = pool.tile([P, D], F32, tag="ln_zc", name="ln_zc")
    nc.vector.tensor_scalar(zc[:], z[:], mean[:], None, op0=ALU.subtract)
    sq = pool.tile([P, D], F32, tag="ln_sq", name="ln_sq")
    nc.vector.tensor_tensor(sq[:], zc[:], zc[:], op=ALU.mult)
    var = pool.tile([P, 1], F32, tag="ln_v", name="ln_v")
    nc.vector.reduce_sum(var[:], sq[:], axis=mybir.AxisListType.X)
    nc.vector.tensor_scalar(var[:], var[:], 1.0 / D, EPS, op0=ALU.mult, op1=ALU.add)
    nc.scalar.activation(var[:], var[:], AF.Sqrt)
    nc.vector.reciprocal(var[:], var[:])
    nc.vector.tensor_scalar(zc[:], zc[:], var[:], None, op0=ALU.mult)
    nc.vector.tensor_tensor(zc[:], zc[:], g[:], op=ALU.mult)
    nc.vector.tensor_tensor(out[:], zc[:], b[:], op=ALU.add)


def make_inputs(inputs):
    """Build per-core in_maps from full inputs."""
    x = np.asarray(inputs["x"], np.float32)
    ipw = np.asarray(inputs["in_proj_w"], np.float32)
    ipb = np.asarray(inputs["in_proj_b"], np.float32)
    out_w = np.asarray(inputs["out_w"], np.float32)
    out_b = np.asarray(inputs["out_b"], np.float32)
    ln1_g = np.asarray(inputs["ln1_g"], np.float32)
    ln1_b = np.asarray(inputs["ln1_b"], np.float32)
    gate_w = np.asarray(inputs["gate_w"], np.float32)
    gate_b = np.asarray(inputs["gate_b"], np.float32)
    w1 = np.asarray(inputs["w1"], np.float32)
    b1 = np.asarray(inputs["b1"], np.float32)
    w2 = np.asarray(inputs["w2"], np.float32)
    b2 = np.asarray(inputs["b2"], np.float32)
    ln2_g = np.asarray(inputs["ln2_g"], np.float32)
    ln2_b = np.asarray(inputs["ln2_b"], np.float32)

    xt = np.ascontiguousarray(np.transpose(x, (0, 2, 1)))  # [B, D, S]
    xf = x.reshape(N, D)
    bc = lambda v: np.ascontiguousarray(np.broadcast_to(v, (P, v.shape[0]))).astype(np.float32)
    common = {
        "xt": xt, "owt": np.ascontiguousarray(out_w.T), "ob128": bc(out_b),
        "lg1": bc(ln1_g), "lb1": bc(ln1_b), "lg2": bc(ln2_g), "lb2": bc(ln2_b),
        "gwt": np.ascontiguousarray(gate_w.T), "gb128": bc(gate_b),
        "ident": np.eye(P, dtype=np.float32),
        "ones128": np.ones((P, 1), np.float32),
    }
    maps = []
    for c in range(NC):
        r = slice(P * c, P * (c + 1))
        oh = np.zeros((P, E), np.float32); oh[:, c] = 1.0
        m = dict(common)
        m.update({
            "xs": xf[TOK * c: TOK * (c + 1)],
            "wqt": np.ascontiguousarray(ipw[r].T),
            "wkt": np.ascontiguousarray(ipw[D:][r].T),
            "wvt": np.ascontiguousarray(ipw[2 * D:][r].T),
            "bq": ipb[r][:, None].copy(), "bk": ipb[D:][r][:, None].copy(),
            "bv": ipb[2 * D:][r][:, None].copy(),
            "w1t": np.ascontiguousarray(w1[c].T).astype(ml_dtypes.bfloat16),
            "b1e": np.ascontiguousarray(b1[c].reshape(FF // P, P).T),
            "w2t": np.ascontiguousarray(w2[c].T).astype(ml_dtypes.bfloat16),
            "b2e128": bc(b2[c]),
            "onehot": oh,
        })
        maps.append(m)
    return maps


def run(inputs, trace=False):
    if "nc" not in _CACHE:
        _CACHE["nc"] = build_nc()
    nc = _CACHE["nc"]
    maps = make_inputs(inputs)
    res = run_bass_kernel_spmd(nc, maps, core_ids=list(range(NC)), trace=trace)
    x2 = np.concatenate([res.results[c]["x2s"] for c in range(NC)], axis=0)
    lb = np.float32(res.results[0]["lb"][0, 0])
    return (x2.reshape(B, S, D), lb), res


def kernel(**inputs):
    out, _ = run(inputs, trace=False)
    return out
